# revision 38
# baseline (speedup 1.0000x reference)
"""Causal single-head attention (B=16, T=2048, C=1024, H=64) on 8 TRN2 NeuronCores.

Strategy (software-pipelined bf16, ~113.5us vs 117.2us baseline):
- Data-parallel over batch: 2 batches per core, weights replicated.
- Host passes x pre-transposed per batch (xT: [C, T]).
- Projections: packed [Wq.T | Wk.T] stationary -> QKT [128, T]; Wv.T -> VT
  [64, T]; V^T transposed to V natural via PE transpose.
- Attention transposed: S^T[k,q] = KT_blk.T @ QT, one full-width matmul per
  k-chunk pair into a [128,1024] PSUM tile (masked diagonal regions are
  computed as real scores); one large exp per pair on ACT; the causal mask is
  a 128/256-wide bf16 multiply on the vector engine; PV pieces are full-width
  (>=256 cols) so PE weight loads stay hidden under compute.
- O'^T[96,q] = [V|1].T @ P accumulated over k-chunks; row 64 = softmax denom.
  Final PE transpose, reciprocal + scale, DMA out.
- Schedule: 8 phases (batch x t-slice). proj(p+1) matmuls and the epilogue of
  phase p-1 are interleaved between attn(p)'s S/PV pairs so the PE stream
  always has ready work while ACT computes exp and DVE applies masks. All x
  DMAs are prefetched at t=0 on the sync/gpsimd queues (first slice split
  into per-chunk pieces); consts ride scalar's short queue, wqk chunk 0
  first since it gates the first matmul; y writeback DMAs alternate
  sync/gpsimd with a deep y-tile pool so queue backlog never blocks compute.
"""
import os
import sys

for _p in ("/opt/trn_rl_repo", "/root/.axon_site/_ro/trn_rl_repo"):
    if os.path.isdir(_p) and _p not in sys.path:
        sys.path.insert(0, _p)

import numpy as np
import ml_dtypes
import concourse.bacc as bacc
import concourse.mybir as mybir
from concourse.tile import TileContext
from concourse import bass_utils

F32 = mybir.dt.float32
F32R = mybir.dt.float32r
BF16 = mybir.dt.bfloat16
EXP = mybir.ActivationFunctionType.Exp

B, T, C, H = 16, 2048, 1024, 64
NCORES = 8
BPC = B // NCORES          # batches per core
NTS = T // 512             # 4 t/q slices of 512
NCH = C // 128             # 8 contraction chunks
NKC = T // 128             # 16 k chunks
NPH = BPC * NTS            # 8 phases

LAST_EXEC_TIME_NS = None
LAST_RESULTS = None


def build():
    nc = bacc.Bacc(trn_type="TRN2")
    xt = nc.dram_tensor("xt", [BPC, C, T], BF16, kind="ExternalInput")
    wqk = nc.dram_tensor("wqk", [C, 128], BF16, kind="ExternalInput")
    wv = nc.dram_tensor("wv", [C, H], BF16, kind="ExternalInput")
    mask2 = nc.dram_tensor("mask2", [128, 256], BF16, kind="ExternalInput")
    ident = nc.dram_tensor("ident", [128, 128], F32R, kind="ExternalInput")
    ident_bf = nc.dram_tensor("ident_bf", [64, 64], BF16, kind="ExternalInput")
    y = nc.dram_tensor("y", [BPC, T, H], F32, kind="ExternalOutput")

    with TileContext(nc) as tc:
        with tc.tile_pool(name="const", bufs=1) as const, \
             tc.tile_pool(name="xpool", bufs=BPC) as xpool, \
             tc.tile_pool(name="qktp", bufs=2) as qktp, \
             tc.tile_pool(name="vtp", bufs=2) as vtp, \
             tc.tile_pool(name="ktp", bufs=2) as ktp, \
             tc.tile_pool(name="vbigp", bufs=1) as vbigp, \
             tc.tile_pool(name="ptp", bufs=8) as ptp, \
             tc.tile_pool(name="osbp", bufs=3) as osbp, \
             tc.tile_pool(name="recp", bufs=8) as recp, \
             tc.tile_pool(name="yp", bufs=40) as yp, \
             tc.tile_pool(name="pspair", bufs=2, space="PSUM") as pspair, \
             tc.tile_pool(name="psop", bufs=1, space="PSUM") as psop, \
             tc.tile_pool(name="pssm", bufs=3, space="PSUM") as pssm:

            # ---- constants on scalar's (short) queue; sync/gpsimd start x
            # transfers immediately. wqk first: it gates the first matmul.
            wqk_sb = const.tile([128, NCH * 128], BF16, name="wqk_sb")
            nc.scalar.dma_start(wqk_sb[:, 0:128], wqk[0:128, :])
            nc.scalar.dma_start(
                wqk_sb[:, 128:].rearrange("p (a f) -> p a f", f=128),
                wqk[128:, :].rearrange("(a p) f -> p a f", p=128))
            id_bf = const.tile([64, 64], BF16, name="id_bf")
            nc.scalar.dma_start(id_bf[:], ident_bf[:])
            wv_sb = const.tile([128, NCH * H], BF16, name="wv_sb")
            nc.scalar.dma_start(
                wv_sb[:].rearrange("p (a f) -> p a f", f=H),
                wv[:].rearrange("(a p) f -> p a f", p=128))
            mask_sb = const.tile([128, 256], BF16, name="mask_sb")
            nc.scalar.dma_start(mask_sb[:], mask2[:])
            id_sb = const.tile([128, 128], F32R, name="id_sb")
            nc.scalar.dma_start(id_sb[:], ident[:])

            # ---- per-batch persistent tiles; vbig denom col via memset ----
            qkts, vts, kts, vbigs = [], [], [], []
            for b in range(BPC):
                qkts.append(qktp.tile([128, T], BF16, name=f"qkt{b}", tag="qkt"))
                vts.append(vtp.tile([64, T], BF16, name=f"vt{b}", tag="vt"))
                kts.append(ktp.tile([64, T], BF16, name=f"kt{b}", tag="kt"))
                vbig = vbigp.tile([128, NKC * 96], BF16, name=f"vbig{b}",
                                  tag=f"vbig{b}")
                for i in range(NKC):
                    nc.gpsimd.memset(vbig[:, 96 * i + H:96 * i + H + 1], 1.0)
                vbigs.append(vbig)

            # ---- prefetch ALL x tiles on sync/gpsimd queues ----
            x_engines = [nc.sync, nc.gpsimd]
            xgs = {}
            n_dma = 0
            for b in range(BPC):
                for ts in range(NTS):
                    for g in range(4):
                        xg = xpool.tile([128, 2 * 512], BF16,
                                        name=f"xg{b}_{ts}_{g}",
                                        tag=f"xg{ts}_{g}")
                        if b == 0 and ts == 0:
                            # split the startup-critical tiles into per-chunk
                            # DMAs so the first projection starts ASAP
                            for h in range(2):
                                src = xt[b, 256 * g + 128 * h:
                                         256 * g + 128 * (h + 1), 0:512]
                                x_engines[n_dma % 2].dma_start(
                                    xg[:, 512 * h:512 * (h + 1)], src)
                                n_dma += 1
                        else:
                            src = xt[b, 256 * g:256 * (g + 1),
                                     512 * ts:512 * (ts + 1)].rearrange(
                                         "(a p) t -> p a t", p=128)
                            dst = xg[:].rearrange("p (a t) -> p a t", t=512)
                            x_engines[n_dma % 2].dma_start(dst, src)
                            n_dma += 1
                        xgs[(b, ts, g)] = xg

            y_engines = [nc.sync, nc.gpsimd]
            state = {"n_y": 0}

            def proj_ops(p):
                """Ordered op-closures for projections of phase p."""
                b, ts = divmod(p, NTS)
                qkt, vt, kt, vbig = qkts[b], vts[b], kts[b], vbigs[b]
                xts = [xgs[(b, ts, c // 2)][:, 512 * (c % 2):512 * (c % 2 + 1)]
                       for c in range(NCH)]
                cell = {}
                ops = []

                def qk_mm(c):
                    if c == 0:
                        cell["qk_ps"] = pssm.tile([128, 512], F32,
                                                  name="qk_ps", tag="pssm")
                    nc.tensor.matmul(cell["qk_ps"][:],
                                     wqk_sb[:, 128 * c:128 * (c + 1)],
                                     xts[c], start=(c == 0),
                                     stop=(c == NCH - 1))
                for c in range(NCH):
                    ops.append(lambda c=c: qk_mm(c))

                def qk_out():
                    nc.vector.tensor_copy(qkt[:, 512 * ts:512 * (ts + 1)],
                                          cell["qk_ps"][:])
                    nc.scalar.dma_start(kt[:, 512 * ts:512 * (ts + 1)],
                                        qkt[64:128, 512 * ts:512 * (ts + 1)])
                ops.append(qk_out)

                def v_mm(c):
                    if c == 0:
                        cell["v_ps"] = pssm.tile([64, 512], F32,
                                                 name="v_ps", tag="pssm")
                    nc.tensor.matmul(cell["v_ps"][:],
                                     wv_sb[:, H * c:H * (c + 1)],
                                     xts[c], start=(c == 0),
                                     stop=(c == NCH - 1))
                for c in range(NCH):
                    ops.append(lambda c=c: v_mm(c))

                def v_out():
                    nc.vector.tensor_copy(vt[:, 512 * ts:512 * (ts + 1)],
                                          cell["v_ps"][:])
                ops.append(v_out)

                def vtr(i):
                    vtr_ps = pssm.tile([128, H], BF16, name="vtr_ps", tag="pssm")
                    nc.tensor.transpose(vtr_ps[:], vt[:, 128 * i:128 * (i + 1)],
                                        id_bf[:])
                    nc.vector.tensor_copy(vbig[:, 96 * i:96 * i + H], vtr_ps[:])
                for i in range(4 * ts, 4 * ts + 4):
                    ops.append(lambda i=i: vtr(i))
                return ops

            def epilogue_ops(p, o_ps):
                """Finish phase p: transpose O back, normalize, DMA out."""
                b, ts = divmod(p, NTS)
                j = ts
                cell = {}
                ops = []

                def osb():
                    cell["o_sb"] = osbp.tile([96, 512], F32R, name="o_sb",
                                             tag="osb")
                    nc.vector.tensor_copy(cell["o_sb"][:], o_ps[:])
                ops.append(osb)

                def fin(s):
                    o_sb = cell["o_sb"]
                    f_ps = pssm.tile([128, 96], F32R, name="f_ps", tag="pssm")
                    nc.tensor.transpose(f_ps[:], o_sb[:, 128 * s:128 * (s + 1)],
                                        id_sb[0:96, 0:96])
                    rec = recp.tile([128, 1], F32, name="rec", tag="rec")
                    nc.vector.reciprocal(rec[:], f_ps[:, H:H + 1])
                    y_t = yp.tile([128, H], F32, name="y_t", tag="yt")
                    nc.vector.tensor_scalar_mul(y_t[:], f_ps[:, 0:H], rec[:])
                    q0 = 512 * j + 128 * s
                    y_engines[state["n_y"] % 2].dma_start(
                        y[b, q0:q0 + 128, :], y_t[:])
                    state["n_y"] += 1
                for s in range(4):
                    ops.append(lambda s=s: fin(s))
                return ops

            def epilogue_half(p, o_ps, half):
                """Half-epilogue for the last phase: cols [256h, 256h+256)."""
                b, ts = divmod(p, NTS)
                j = ts
                cell = {}
                ops = []

                def osb():
                    cell["o_sb"] = osbp.tile([96, 256], F32R, name="o_sbh",
                                             tag="osbh")
                    nc.vector.tensor_copy(cell["o_sb"][:],
                                          o_ps[:, 256 * half:256 * half + 256])
                ops.append(osb)

                def fin(s):
                    o_sb = cell["o_sb"]
                    lc = 128 * (s % 2)
                    f_ps = pssm.tile([128, 96], F32R, name="f_ps", tag="pssm")
                    nc.tensor.transpose(f_ps[:], o_sb[:, lc:lc + 128],
                                        id_sb[0:96, 0:96])
                    rec = recp.tile([128, 1], F32, name="rec", tag="rec")
                    nc.vector.reciprocal(rec[:], f_ps[:, H:H + 1])
                    y_t = yp.tile([128, H], F32, name="y_t", tag="yt")
                    nc.vector.tensor_scalar_mul(y_t[:], f_ps[:, 0:H], rec[:])
                    q0 = 512 * j + 128 * s
                    y_engines[state["n_y"] % 2].dma_start(
                        y[b, q0:q0 + 128, :], y_t[:])
                    state["n_y"] += 1
                for s in (2 * half, 2 * half + 1):
                    ops.append(lambda s=s: fin(s))
                return ops

            # ---- phase 0 prologue ----
            for op in proj_ops(0):
                op()

            pending = []   # ops from epilogue(p-1) + proj(p+1) to interleave
            for p in range(NPH):
                b, ts = divmod(p, NTS)
                j = ts
                nck = 4 * j + 4
                qkt, kt, vbig = qkts[b], kts[b], vbigs[b]
                q_mv = qkt[0:64, 512 * j:512 * (j + 1)]

                if p + 1 < NPH:
                    pending = pending + proj_ops(p + 1)
                quota = -(-max(1, len(pending)) // (nck // 2))  # ceil

                o_ps = None
                pv_q = []     # clean PV pieces: (p_pair, half, i, lo, hi)
                pv_tri = []   # mask-gated PV pieces, flushed at row end

                def emit_pv(entry, start, stop):
                    pp, half, ii, lo, hi = entry
                    nc.tensor.matmul(
                        o_ps[:, lo:hi],
                        vbig[:, 96 * ii:96 * (ii + 1)],
                        pp[:, 512 * half + lo:512 * half + hi],
                        start=start, stop=stop)

                for u in range(nck // 2):
                    i0, i1 = 2 * u, 2 * u + 1
                    d0 = i0 - 4 * j
                    o_e = 128 * d0 if d0 > 0 else 0
                    s_pair = pspair.tile([128, 1024], F32, name="s_pair",
                                         tag="s_pair")
                    nc.tensor.matmul(s_pair[:, o_e:512],
                                     kt[:, 128 * i0:128 * (i0 + 1)],
                                     q_mv[:, o_e:512], start=True, stop=True)
                    nc.tensor.matmul(s_pair[:, 512 + o_e:1024],
                                     kt[:, 128 * i1:128 * (i1 + 1)],
                                     q_mv[:, o_e:512], start=True, stop=True)
                    p_pair = ptp.tile([128, 1024], BF16, name="p_pair",
                                      tag="p_pair")
                    if o_e == 0:
                        nc.scalar.activation(p_pair[:], s_pair[:], EXP,
                                             scale=0.125)
                    else:
                        pv_view = p_pair[:].rearrange(
                            "p (a t) -> p a t", t=512)[:, :, o_e:512]
                        sv_view = s_pair[:].rearrange(
                            "p (a t) -> p a t", t=512)[:, :, o_e:512]
                        nc.scalar.activation(pv_view, sv_view, EXP,
                                             scale=0.125)
                    if d0 >= 0:
                        # masked regions -> zeros (off the PV critical path:
                        # only the small tri-PV pieces below wait on these)
                        nc.vector.tensor_mul(
                            p_pair[:, 128 * d0:128 * d0 + 128],
                            p_pair[:, 128 * d0:128 * d0 + 128],
                            mask_sb[:, 128:256])
                        nc.vector.tensor_mul(
                            p_pair[:, 512 + o_e:512 + o_e + 256],
                            p_pair[:, 512 + o_e:512 + o_e + 256],
                            mask_sb[:, 0:256])
                        # full-width PV pieces (>=256 cols keeps the PE
                        # sequencer from starving on weight loads); the odd
                        # half's dead zone [o_e, 128d0+128) is mask-zeroed.
                        pv_q.append((p_pair, 0, i0, o_e, 512))
                        pv_q.append((p_pair, 1, i1, o_e, 512))
                    else:
                        pv_q.append((p_pair, 0, i0, 0, 512))
                        pv_q.append((p_pair, 1, i1, 0, 512))

                    # interleave pipelined work from other phases
                    for _ in range(quota):
                        if pending:
                            pending.pop(0)()

                    while len(pv_q) > 4:
                        if o_ps is None:
                            o_ps = psop.tile([96, 512], F32, name="o_ps",
                                             tag="o_ps")
                        e = pv_q.pop(0)
                        emit_pv(e, start=(e[2] == 0), stop=False)
                while pv_q:
                    if o_ps is None:
                        o_ps = psop.tile([96, 512], F32, name="o_ps",
                                         tag="o_ps")
                    e = pv_q.pop(0)
                    emit_pv(e, start=(e[2] == 0), stop=(len(pv_q) == 0))
                    if p == NPH - 1 and e[2] == nck - 3:
                        # tail shave: o_ps cols [0,256) are final after chunk
                        # nck-3 (the last two chunks only write [256,512)), so
                        # start the first half of the final epilogue now.
                        for op in epilogue_half(p, o_ps, 0):
                            op()
                # finish any leftover interleaved work before queueing epilogue
                while pending:
                    pending.pop(0)()
                if p == NPH - 1:
                    pending = epilogue_half(p, o_ps, 1)
                else:
                    pending = epilogue_ops(p, o_ps)
            while pending:
                pending.pop(0)()

    nc.finalize()
    return nc


_NC_CACHE = None


def _get_nc():
    global _NC_CACHE
    if _NC_CACHE is None:
        _NC_CACHE = build()
    return _NC_CACHE


def _make_mask2():
    # mask2[p, m] = 1.0 iff m >= p + 128
    # even diag chunks use mask2[:, 128:256] (=> c' >= p, c' = m-128)
    # odd diag chunks use mask2[:, 0:256]    (=> c' >= p + 128, c' = m)
    p = np.arange(128)[:, None]
    m = np.arange(256)[None, :]
    return (m >= p + 128).astype(np.float32)


def kernel(x, Wk, Wq, Wv, _trace=False, _trace_kwargs=None):
    global LAST_EXEC_TIME_NS, LAST_RESULTS
    x = np.ascontiguousarray(np.asarray(x, dtype=np.float32))
    Wk = np.asarray(Wk, dtype=np.float32)
    Wq = np.asarray(Wq, dtype=np.float32)
    Wv = np.asarray(Wv, dtype=np.float32)

    wqk = np.ascontiguousarray(
        np.concatenate([Wq.T, Wk.T], axis=1)).astype(ml_dtypes.bfloat16)  # [C, 128]
    wv = np.ascontiguousarray(Wv.T).astype(ml_dtypes.bfloat16)            # [C, H]
    mask_np = _make_mask2().astype(ml_dtypes.bfloat16)
    ident = np.eye(128, dtype=np.float32)
    ident_bf = np.eye(64, dtype=ml_dtypes.bfloat16)

    in_maps = []
    for core in range(NCORES):
        xb = x[BPC * core:BPC * (core + 1)]                 # [2, T, C]
        xtb = np.ascontiguousarray(xb.transpose(0, 2, 1)).astype(ml_dtypes.bfloat16)
        in_maps.append({"xt": xtb, "wqk": wqk, "wv": wv, "mask2": mask_np,
                        "ident": ident, "ident_bf": ident_bf})

    nc = _get_nc()
    kwargs = {}
    if _trace:
        kwargs["trace"] = True
        if _trace_kwargs:
            kwargs.update(_trace_kwargs)
    res = bass_utils.run_bass_kernel_spmd(nc, in_maps, core_ids=list(range(NCORES)),
                                          **kwargs)
    LAST_EXEC_TIME_NS = res.exec_time_ns
    LAST_RESULTS = res

    out = np.empty((B, T, H), dtype=np.float32)
    for core in range(NCORES):
        out[BPC * core:BPC * (core + 1)] = res.results[core]["y"]
    return out
